# revision 28
# baseline (speedup 1.0000x reference)
"""Trainium2 Bass kernel for nn_PhaseAdaptiveInput (embedding lookup).

out[b] = act(sum_f W[feature_indices[b,f], bucket(b)*256:(bucket(b)+1)*256] + bias_bucket)
with bucket(b) = ply[b] // 7 and act(x) = clip(x,0,1)^2 * 255/256.

Strategy (8 NeuronCores, data parallel over samples, bucket-sharded):
  - Samples grouped by bucket host-side; core k gets bucket k's samples
    (~1024). All cores run ONE static SPMD program; the per-core bucket
    enters via the W input, a numpy VIEW of the flat table starting at
    element 256*k.
  - Gathers are organized as per-(128-sample block, table half) segments,
    statically sized to the max count across the 8 cores (rounded to 128),
    split into sub-calls of <= 768 indices. Sub-calls are issued round-robin
    on 4 SWDGE queues so descriptor generation runs concurrently on 4 Q7
    core pairs (the single-queue Q7 descgen rate of ~8.5 ns/idx is the
    bottleneck otherwise).
  - Rows split by half (row < 32768 vs >= 32768) because gather indices are
    signed int16; the upper half uses a W view offset by 32768 rows.
  - Per 128-slot chunk a 0/1 mask [slot, sample] is built on DVE (owner-id
    vs iota is_equal) and chunk matmuls (float32r) accumulate per-sample
    sums into a per-block PSUM tile.
  - Epilogue per block: +bias, clip to [0,1], square, *255/256, DMA out.

The program is compiled on first call, specialized to the input's segment
sizes; recompiled only if a later input needs bigger segments.

Self-contained: hardcodes all shapes for the 8192x32 / 65536x2048 problem.
"""
import sys
import numpy as np

for _p in ("/opt/trn_rl_repo", "/root/.axon_site/_ro/trn_rl_repo"):
    if _p not in sys.path:
        sys.path.append(_p)

# ---------------------------------------------------------------- constants
BATCH = 8192
NFEAT = 32
NROWS = 65536
COUNT = 8
ODIM = 256
BUCKET_SIZE = 7
ACT_SCALE = 255.0 / 256.0
ROW_STRIDE = 2048          # f32 elements per table row
NH = 32768                 # rows per int16-addressable half
SUBCAP = 2304              # max idxs per gather sub-call (multi-packet)
NQUEUES = 4
W_LEN = (2 * NH - 1) * ROW_STRIDE + ODIM   # per-core view length
GATHER_BUFS = 8
MASK_BUFS = 6

_compiled = None           # (nc, layout)


def _plan_layout(seg_sizes):
    """seg_sizes: list over (block, half) in stream order of static sizes
    (multiples of 128, possibly 0). Returns the static call plan."""
    calls = []  # (block, half, size, icol_off, chunk_off, queue)
    icol_off = 0
    chunk_off = 0
    ci = 0
    for b in range(len(seg_sizes) // 2):
        for h in (0, 1):
            seg = seg_sizes[2 * b + h]
            if seg == 0:
                continue
            # near-uniform sub-call sizes (multiples of 128) so concurrent
            # queue groups don't wait on one long straggler call
            n_sub = -(-seg // SUBCAP)
            units = seg // 128
            per, rem = divmod(units, n_sub)
            sizes = [(per + 1) * 128] * rem + [per * 128] * (n_sub - rem)
            for size in sizes:
                calls.append((b, h, size, icol_off, chunk_off, ci % NQUEUES))
                icol_off += size // 16
                chunk_off += size // 128
                ci += 1
    return calls


def _build_program(seg_sizes, full_flags):
    import concourse.bacc as bacc
    import concourse.bass as bass
    import concourse.mybir as mybir
    import concourse.tile as tile
    from concourse.library_config import mlp

    F32 = mybir.dt.float32
    BF16 = mybir.dt.bfloat16

    calls = _plan_layout(seg_sizes)
    nblocks = len(seg_sizes) // 2
    ncalls = len(calls)
    total_icol = sum(c[2] for c in calls) // 16
    total_chunks = sum(c[2] for c in calls) // 128
    # first/last chunk index per block (for PSUM start/stop)
    blk_first = {}
    blk_last = {}
    for b, h, size, io, co, q in calls:
        for j in range(size // 128):
            ch = co + j
            blk_first.setdefault(b, ch)
            blk_last[b] = ch
    used_blocks = sorted(blk_first)

    nc = bacc.Bacc("TRN2", target_bir_lowering=False, debug=False,
                   num_swdge_queues=NQUEUES)
    w = nc.dram_tensor("w", [W_LEN], BF16, kind="ExternalInput")
    idxs_d = nc.dram_tensor("idxs", [128, max(16, total_icol)],
                            mybir.dt.int16, kind="ExternalInput")
    owners_d = nc.dram_tensor("owners", [128, max(8, total_chunks)], BF16,
                              kind="ExternalInput")
    counts_d = nc.dram_tensor("counts", [1, max(32, ncalls)], mybir.dt.int32,
                              kind="ExternalInput")
    zeros_d = nc.dram_tensor("zeros", [128, SUBCAP // 128, ODIM], BF16,
                             kind="ExternalInput")
    bias_d = nc.dram_tensor("biasrep", [128, ODIM], F32, kind="ExternalInput")
    iota_d = nc.dram_tensor("iota", [128, 128], BF16, kind="ExternalInput")
    out_d = nc.dram_tensor("out", [nblocks * 128, ODIM], F32,
                           kind="ExternalOutput")
    wt = w[:].tensor

    with tile.TileContext(nc) as tc:
        with tc.tile_pool(name="const", bufs=1) as cpool, \
             tc.tile_pool(name="gather", bufs=GATHER_BUFS) as gpool, \
             tc.tile_pool(name="mask", bufs=MASK_BUFS) as mpool, \
             tc.tile_pool(name="acts", bufs=2) as apool, \
             tc.tile_pool(name="psum", bufs=4, space="PSUM") as pspool:
            nc.gpsimd.load_library(mlp)
            idx_t = cpool.tile([128, max(16, total_icol)], mybir.dt.int16,
                               tag="idx")
            own_t = cpool.tile([128, max(8, total_chunks)], BF16, tag="own")
            cnt_t = cpool.tile([1, max(32, ncalls)], mybir.dt.int32,
                               tag="cnt")
            bias_t = cpool.tile([128, ODIM], F32, tag="bias")
            iota_t = cpool.tile([128, 128], BF16, tag="iota")
            nc.sync.dma_start(idx_t[:, :], idxs_d[:, :])
            nc.sync.dma_start(own_t[:, :], owners_d[:, :])
            nc.sync.dma_start(cnt_t[:, :], counts_d[:, :])
            nc.sync.dma_start(bias_t[:, :], bias_d[:, :])
            nc.sync.dma_start(iota_t[:, :], iota_d[:, :])
            zero_s = cpool.tile([128, 1], F32, tag="zs")
            negone_s = cpool.tile([128, 1], F32, tag="ns")
            nc.vector.memset(zero_s[:], 0.0)
            nc.vector.memset(negone_s[:], -1.0)

            psum_tiles = {}
            ncold = min(GATHER_BUFS, ncalls)
            for ci, (b, h, size, io, co, q) in enumerate(calls):
                schunk = size // 128
                dst = gpool.tile([128, SUBCAP // 128, ODIM], BF16, tag="dst")
                if ci < ncold and not full_flags[ci]:
                    # first use of this pool buffer gathers fewer rows than
                    # the static size: zero-fill via HWDGE DMA so the unset
                    # slots hold 0.0 rather than uninitialized SBUF (0 x NaN
                    # would poison the matmul). Later reuses inherit valid
                    # bf16 table values, so only first uses need this.
                    nc.sync.dma_start(dst[:], zeros_d[:, :, :])
                if full_flags[ci]:
                    # call is full on every core: immediate count, no
                    # value_load in the dispatch stream
                    cnt = size
                else:
                    cnt = nc.gpsimd.value_load(cnt_t[:1, ci:ci + 1])
                w_view = bass.AP(tensor=wt, offset=h * NH * ROW_STRIDE,
                                 ap=[(ROW_STRIDE, NH), (1, ODIM)])
                nc.gpsimd.dma_gather(
                    dst[:, :schunk, :], w_view,
                    idx_t[:, io:io + size // 16],
                    size, cnt, ODIM, elem_step=ROW_STRIDE,
                    single_packet=False, queue_num=q)

                mask = mpool.tile([128, SUBCAP // 128, 128], BF16, tag="mask")
                own_bc = own_t[:, co:co + schunk] \
                    .unsqueeze(2).to_broadcast([128, schunk, 128])
                iota_bc = iota_t[:, :].unsqueeze(1) \
                    .to_broadcast([128, schunk, 128])
                nc.vector.tensor_tensor(mask[:, :schunk, :], own_bc, iota_bc,
                                        mybir.AluOpType.is_equal)

                if blk_first[b] == co:
                    psum_tiles[b] = pspool.tile([128, ODIM], F32, tag="ps",
                                                name=f"ps{b}")
                pt = psum_tiles[b]
                for j in range(schunk):
                    ch = co + j
                    nc.tensor.matmul(pt[:],
                                     lhsT=mask[:, j, :],
                                     rhs=dst[:, j, :],
                                     start=(ch == blk_first[b]),
                                     stop=(ch == blk_last[b]))
                    if ch == blk_last[b]:
                        # clip(x,0,1) == relu(x) - relu(x-1); then
                        # square-and-scale via Square(d*sqrt(s)) = d^2*s.
                        # relu/square run on the (otherwise idle) ACT engine.
                        act = apool.tile([128, ODIM], F32, tag="act")
                        r1 = apool.tile([128, ODIM], F32, tag="r1")
                        nc.vector.tensor_add(act[:], pt[:], bias_t[:])
                        nc.scalar.activation(
                            r1[:], act[:],
                            mybir.ActivationFunctionType.Relu,
                            bias=negone_s[:, :1])
                        nc.scalar.activation(
                            act[:], act[:],
                            mybir.ActivationFunctionType.Relu,
                            bias=zero_s[:, :1])
                        nc.vector.tensor_sub(act[:], act[:], r1[:])
                        nc.scalar.activation(
                            act[:], act[:],
                            mybir.ActivationFunctionType.Square,
                            bias=zero_s[:, :1],
                            scale=float(ACT_SCALE ** 0.5))
                        nc.sync.dma_start(
                            out_d[b * 128:(b + 1) * 128, :], act[:])
            # blocks with zero static size still need defined output rows
            for b in range(nblocks):
                if b not in blk_first:
                    act = apool.tile([128, ODIM], F32, tag="act")
                    nc.vector.memset(act[:], 0.0)
                    nc.sync.dma_start(out_d[b * 128:(b + 1) * 128, :], act[:])
    nc.compile()
    return nc


def _host_prep(feature_indices, ply):
    """Returns (seg_sizes, per-core data dicts, samp_ids)."""
    fi = np.asarray(feature_indices, dtype=np.int64)
    plyv = np.asarray(ply, dtype=np.int64)
    bucket = np.clip(plyv // BUCKET_SIZE, 0, COUNT - 1)

    samp_ids = []
    core_ents = []  # per core: list over blocks of (idx_h0, own_h0, idx_h1, own_h1)
    nmax = 0
    for k in range(COUNT):
        samp = np.nonzero(bucket == k)[0]
        samp_ids.append(samp)
        nmax = max(nmax, len(samp))
    nblocks = max(1, -(-nmax // 128))

    for k in range(COUNT):
        samp = samp_ids[k]
        n = len(samp)
        rows = fi[samp]  # [n, 32]
        blocks = []
        for b in range(nblocks):
            lo, hi = b * 128, min(n, b * 128 + 128)
            if hi > lo:
                r = rows[lo:hi]
                owner = np.repeat(np.arange(hi - lo), NFEAT)
                rflat = r.reshape(-1)
                half = rflat >= NH
                per_half = []
                for h in (0, 1):
                    sel = np.nonzero(half == bool(h))[0]
                    per_half.append((rflat[sel] - h * NH,
                                     owner[sel].astype(np.float32)))
                blocks.append(per_half)
            else:
                z = (np.zeros(0, np.int64), np.zeros(0, np.float32))
                blocks.append([z, z])
        core_ents.append(blocks)

    # static segment sizes: max over cores, rounded up to 128
    seg_sizes = []
    for b in range(nblocks):
        for h in (0, 1):
            m = max(len(core_ents[k][b][h][0]) for k in range(COUNT))
            seg_sizes.append(-(-m // 128) * 128)

    calls = _plan_layout(seg_sizes)
    ncalls = len(calls)
    total_icol = sum(c[2] for c in calls) // 16
    total_chunks = sum(c[2] for c in calls) // 128
    import ml_dtypes
    iota = np.broadcast_to(np.arange(128, dtype=np.float32),
                           (128, 128)).astype(ml_dtypes.bfloat16)

    zeros = np.zeros((128, SUBCAP // 128, ODIM), ml_dtypes.bfloat16)
    cores = []
    for k in range(COUNT):
        idx_arr = np.full((128, max(16, total_icol)), -1, np.int16)
        own_arr = np.full((128, max(8, total_chunks)), -1.0,
                          ml_dtypes.bfloat16)
        cnt_arr = np.zeros((1, max(32, ncalls)), np.int32)
        for ci, (b, h, size, io, co, q) in enumerate(calls):
            ents, owns = core_ents[k][b][h]
            seg_off = 0
            # offset of this call within its (b,h) segment
            for cj in range(ci - 1, -1, -1):
                pb, ph, psz, _, _, _ = calls[cj]
                if pb == b and ph == h:
                    seg_off += psz
                else:
                    break
            seg = ents[seg_off:seg_off + size]
            osg = owns[seg_off:seg_off + size]
            m = len(seg)
            cnt_arr[0, ci] = m
            if m == 0:
                continue
            col = np.full(size, -1, np.int64)
            ocol = np.full(size, -1.0, np.float32)
            col[:m] = seg
            ocol[:m] = osg
            wrap = col.reshape(size // 16, 16).T.astype(np.int16)
            idx_arr[:, io:io + size // 16] = np.tile(wrap, (8, 1))
            own_arr[:, co:co + size // 128] = \
                ocol.reshape(size // 128, 128).T.astype(ml_dtypes.bfloat16)
        cores.append({"idxs": idx_arr, "owners": own_arr, "counts": cnt_arr,
                      "zeros": zeros, "iota": iota})
    return seg_sizes, cores, samp_ids


def _fallback(feature_indices, ply, W, bias):
    fi = np.asarray(feature_indices, dtype=np.int64)
    plyv = np.asarray(ply, dtype=np.int64)
    bucket = np.clip(plyv // BUCKET_SIZE, 0, COUNT - 1)
    Wr = np.asarray(W, dtype=np.float32).reshape(NROWS, COUNT, ODIM)
    br = np.asarray(bias, np.float32).reshape(COUNT, ODIM)
    out = np.empty((len(plyv), ODIM), np.float32)
    for b in range(len(plyv)):
        acc = Wr[fi[b], bucket[b], :].sum(axis=0) + br[bucket[b]]
        out[b] = np.clip(acc, 0.0, 1.0) ** 2 * ACT_SCALE
    return out


def _make_in_maps(cores, W, bias):
    import ml_dtypes
    wflat = W.reshape(-1).astype(ml_dtypes.bfloat16)
    biasr = bias.reshape(COUNT, ODIM)
    in_maps = []
    for k in range(COUNT):
        m = dict(cores[k])
        m["w"] = wflat[k * ODIM: k * ODIM + W_LEN]
        m["biasrep"] = np.broadcast_to(biasr[k], (128, ODIM)).copy()
        in_maps.append(m)
    return in_maps


def kernel(feature_indices, ply, W, bias):
    global _compiled
    from concourse.bass_utils import run_bass_kernel_spmd

    W = np.ascontiguousarray(np.asarray(W, dtype=np.float32))
    bias = np.asarray(bias, dtype=np.float32)
    seg_sizes, cores, samp_ids = _host_prep(feature_indices, ply)
    if max(len(s) for s in samp_ids) > 4096:
        return _fallback(feature_indices, ply, W, bias)

    calls = _plan_layout(seg_sizes)
    full_flags = tuple(
        all(int(c["counts"][0, ci]) == calls[ci][2] for c in cores)
        for ci in range(len(calls)))
    key = (tuple(seg_sizes), full_flags)
    if _compiled is None or _compiled[1] != key:
        _compiled = (_build_program(seg_sizes, full_flags), key)
    nc = _compiled[0]
    in_maps = _make_in_maps(cores, W, bias)
    res = run_bass_kernel_spmd(nc, in_maps, core_ids=list(range(COUNT)))
    out = np.empty((BATCH, ODIM), np.float32)
    for k in range(COUNT):
        ids = samp_ids[k]
        out[ids] = res.results[k]["out"][: len(ids)]
    return out


# revision 33
# speedup vs baseline: 1.0928x; 1.0928x over previous
"""Trainium2 Bass kernel for nn_PhaseAdaptiveInput (embedding lookup).

out[b] = act(sum_f W[feature_indices[b,f], bucket(b)*256:(bucket(b)+1)*256] + bias_bucket)
with bucket(b) = ply[b] // 7 and act(x) = clip(x,0,1)^2 * 255/256.

Strategy (8 NeuronCores, data parallel over samples, bucket-sharded):
  - Samples grouped by bucket host-side; core k gets bucket k's samples
    (~1024). All cores run ONE static SPMD program; the per-core bucket
    enters via the W input, a numpy VIEW of the (host-converted bf16) flat
    table starting at element 256*k. bf16 halves the gather bytes; rel err
    stays ~5e-3 (budget 2e-2).
  - Gathers are organized as per-(128-sample block, table half) segments,
    statically sized to the max count across the 8 cores (rounded to 128),
    split into near-uniform sub-calls of <= 896 indices (single_packet
    requires <= 63 descriptors/engine). Sub-calls are issued round-robin on
    4 SWDGE queues so descriptor generation runs concurrently on 4 Q7 core
    pairs (the single-queue Q7 descgen rate of ~8.5 ns/static-idx is the
    bottleneck otherwise; per-call cost is linear in static num_idxs, so
    bigger calls do NOT amortize).
  - Rows split by half (row < 32768 vs >= 32768) because gather indices are
    signed int16; the upper half uses a W view offset by 32768 rows.
  - Calls that are full on every core pass num_idxs_reg as an immediate;
    only segment-tail calls need a value_load of the per-core count (the
    count MUST match the Q7-stripped trailing -1 run or ring bookkeeping
    drifts).
  - Per 128-slot chunk a 0/1 bf16 mask [slot, sample] is built on DVE
    (owner-id vs iota is_equal) and chunk matmuls (bf16 -> f32 PSUM)
    accumulate per-sample sums into a per-block PSUM tile.
  - Epilogue per block: +bias on DVE, then clip(x,0,1) = relu(x)-relu(x-1)
    and square*255/256 (= Square(x*sqrt(s))) on the ACT engine.

The program is compiled on first call, specialized to the input's segment
sizes; recompiled only if a later input changes the layout.

Self-contained: hardcodes all shapes for the 8192x32 / 65536x2048 problem.
"""
import sys
import numpy as np

for _p in ("/opt/trn_rl_repo", "/root/.axon_site/_ro/trn_rl_repo"):
    if _p not in sys.path:
        sys.path.append(_p)

# ---------------------------------------------------------------- constants
BATCH = 8192
NFEAT = 32
NROWS = 65536
COUNT = 8
ODIM = 256
BUCKET_SIZE = 7
ACT_SCALE = 255.0 / 256.0
ROW_STRIDE = 2048          # f32 elements per table row
NH = 32768                 # rows per int16-addressable half
SUBCAP = 896               # max idxs per gather sub-call (single_packet: <=63 descs/engine)
NQUEUES = 4
W_LEN = (2 * NH - 1) * ROW_STRIDE + ODIM   # per-core view length
GATHER_BUFS = 20
MASK_BUFS = 12
HEAD_CALLS = 8             # calls whose idxs load via a small head DMA

_compiled = None           # (nc, layout)


def _plan_layout(seg_sizes):
    """seg_sizes: list over (block, half) in stream order of static sizes
    (multiples of 128, possibly 0). Returns the static call plan."""
    calls = []  # (block, half, size, icol_off, chunk_off, queue)
    icol_off = 0
    chunk_off = 0
    ci = 0
    for b in range(len(seg_sizes) // 2):
        for h in (0, 1):
            seg = seg_sizes[2 * b + h]
            if seg == 0:
                continue
            # near-uniform sub-call sizes (multiples of 128) so concurrent
            # queue groups don't wait on one long straggler call
            n_sub = -(-seg // SUBCAP)
            units = seg // 128
            per, rem = divmod(units, n_sub)
            sizes = [(per + 1) * 128] * rem + [per * 128] * (n_sub - rem)
            for size in sizes:
                calls.append((b, h, size, icol_off, chunk_off, ci % NQUEUES))
                icol_off += size // 16
                chunk_off += size // 128
                ci += 1
    return calls


def _build_program(seg_sizes, full_flags):
    import concourse.bacc as bacc
    import concourse.bass as bass
    import concourse.mybir as mybir
    import concourse.tile as tile
    from concourse.library_config import mlp

    F32 = mybir.dt.float32
    BF16 = mybir.dt.bfloat16

    calls = _plan_layout(seg_sizes)
    nblocks = len(seg_sizes) // 2
    ncalls = len(calls)
    total_icol = sum(c[2] for c in calls) // 16
    total_chunks = sum(c[2] for c in calls) // 128
    # first/last chunk index per block (for PSUM start/stop)
    blk_first = {}
    blk_last = {}
    for b, h, size, io, co, q in calls:
        for j in range(size // 128):
            ch = co + j
            blk_first.setdefault(b, ch)
            blk_last[b] = ch
    used_blocks = sorted(blk_first)

    nc = bacc.Bacc("TRN2", target_bir_lowering=False, debug=False,
                   num_swdge_queues=NQUEUES)
    w = nc.dram_tensor("w", [W_LEN], BF16, kind="ExternalInput")
    idxs_d = nc.dram_tensor("idxs", [128, max(16, total_icol)],
                            mybir.dt.int16, kind="ExternalInput")
    owners_d = nc.dram_tensor("owners", [128, max(8, total_chunks)], BF16,
                              kind="ExternalInput")
    counts_d = nc.dram_tensor("counts", [1, max(32, ncalls)], mybir.dt.int32,
                              kind="ExternalInput")
    zeros_d = nc.dram_tensor("zeros", [128, SUBCAP // 128, ODIM], BF16,
                             kind="ExternalInput")
    bias_d = nc.dram_tensor("biasrep", [128, ODIM], F32, kind="ExternalInput")
    iota_d = nc.dram_tensor("iota", [128, 128], BF16, kind="ExternalInput")
    out_d = nc.dram_tensor("out", [nblocks * 128, ODIM], F32,
                           kind="ExternalOutput")
    wt = w[:].tensor

    with tile.TileContext(nc) as tc:
        with tc.tile_pool(name="const", bufs=1) as cpool, \
             tc.tile_pool(name="gather", bufs=GATHER_BUFS) as gpool, \
             tc.tile_pool(name="mask", bufs=MASK_BUFS) as mpool, \
             tc.tile_pool(name="acts", bufs=2) as apool, \
             tc.tile_pool(name="psum", bufs=4, space="PSUM") as pspool:
            nc.gpsimd.load_library(mlp)
            # split the idx load so the first gathers only wait on a small
            # head DMA, not the full index tensor
            c_split = sum(c[2] for c in calls[:HEAD_CALLS]) // 16
            c_split = min(c_split, total_icol)
            idx_a = cpool.tile([128, max(16, c_split)], mybir.dt.int16,
                               tag="idxa")
            idx_b = cpool.tile([128, max(16, total_icol - c_split)],
                               mybir.dt.int16, tag="idxb")
            own_t = cpool.tile([128, max(8, total_chunks)], BF16, tag="own")
            cnt_t = cpool.tile([1, max(32, ncalls)], mybir.dt.int32,
                               tag="cnt")
            bias_t = cpool.tile([128, ODIM], F32, tag="bias")
            iota_t = cpool.tile([128, 128], BF16, tag="iota")
            nc.sync.dma_start(idx_a[:, :c_split], idxs_d[:, :c_split])
            nc.sync.dma_start(own_t[:, :], owners_d[:, :])
            nc.sync.dma_start(cnt_t[:, :], counts_d[:, :])
            nc.sync.dma_start(idx_b[:, :total_icol - c_split],
                              idxs_d[:, c_split:])
            nc.sync.dma_start(bias_t[:, :], bias_d[:, :])
            nc.sync.dma_start(iota_t[:, :], iota_d[:, :])
            zero_s = cpool.tile([128, 1], F32, tag="zs")
            negone_s = cpool.tile([128, 1], F32, tag="ns")
            nc.vector.memset(zero_s[:], 0.0)
            nc.vector.memset(negone_s[:], -1.0)

            psum_tiles = {}
            ncold = min(GATHER_BUFS, ncalls)
            for ci, (b, h, size, io, co, q) in enumerate(calls):
                schunk = size // 128
                dst = gpool.tile([128, SUBCAP // 128, ODIM], BF16, tag="dst")
                if ci < ncold and not full_flags[ci]:
                    # first use of this pool buffer gathers fewer rows than
                    # the static size: zero-fill via HWDGE DMA so the unset
                    # slots hold 0.0 rather than uninitialized SBUF (0 x NaN
                    # would poison the matmul). Later reuses inherit valid
                    # bf16 table values, so only first uses need this.
                    nc.sync.dma_start(dst[:], zeros_d[:, :, :])
                if full_flags[ci]:
                    # call is full on every core: immediate count, no
                    # value_load in the dispatch stream
                    cnt = size
                else:
                    cnt = nc.gpsimd.value_load(cnt_t[:1, ci:ci + 1])
                w_view = bass.AP(tensor=wt, offset=h * NH * ROW_STRIDE,
                                 ap=[(ROW_STRIDE, NH), (1, ODIM)])
                if io < c_split:
                    idx_view = idx_a[:, io:io + size // 16]
                else:
                    idx_view = idx_b[:, io - c_split:io - c_split + size // 16]
                nc.gpsimd.dma_gather(
                    dst[:, :schunk, :], w_view,
                    idx_view,
                    size, cnt, ODIM, elem_step=ROW_STRIDE,
                    single_packet=True, queue_num=q)

                mask = mpool.tile([128, SUBCAP // 128, 128], BF16, tag="mask")
                own_bc = own_t[:, co:co + schunk] \
                    .unsqueeze(2).to_broadcast([128, schunk, 128])
                iota_bc = iota_t[:, :].unsqueeze(1) \
                    .to_broadcast([128, schunk, 128])
                nc.vector.tensor_tensor(mask[:, :schunk, :], own_bc, iota_bc,
                                        mybir.AluOpType.is_equal)

                if blk_first[b] == co:
                    psum_tiles[b] = pspool.tile([128, ODIM], F32, tag="ps",
                                                name=f"ps{b}")
                pt = psum_tiles[b]
                for j in range(schunk):
                    ch = co + j
                    nc.tensor.matmul(pt[:],
                                     lhsT=mask[:, j, :],
                                     rhs=dst[:, j, :],
                                     start=(ch == blk_first[b]),
                                     stop=(ch == blk_last[b]))
                    if ch == blk_last[b]:
                        # clip(x,0,1) == relu(x) - relu(x-1); then
                        # square-and-scale via Square(d*sqrt(s)) = d^2*s.
                        # relu/square run on the (otherwise idle) ACT engine.
                        act = apool.tile([128, ODIM], F32, tag="act")
                        r1 = apool.tile([128, ODIM], F32, tag="r1")
                        nc.vector.tensor_add(act[:], pt[:], bias_t[:])
                        nc.scalar.activation(
                            r1[:], act[:],
                            mybir.ActivationFunctionType.Relu,
                            bias=negone_s[:, :1])
                        nc.scalar.activation(
                            act[:], act[:],
                            mybir.ActivationFunctionType.Relu,
                            bias=zero_s[:, :1])
                        nc.vector.tensor_sub(act[:], act[:], r1[:])
                        nc.scalar.activation(
                            act[:], act[:],
                            mybir.ActivationFunctionType.Square,
                            bias=zero_s[:, :1],
                            scale=float(ACT_SCALE ** 0.5))
                        nc.sync.dma_start(
                            out_d[b * 128:(b + 1) * 128, :], act[:])
            # blocks with zero static size still need defined output rows
            for b in range(nblocks):
                if b not in blk_first:
                    act = apool.tile([128, ODIM], F32, tag="act")
                    nc.vector.memset(act[:], 0.0)
                    nc.sync.dma_start(out_d[b * 128:(b + 1) * 128, :], act[:])
    nc.compile()
    return nc


def _host_prep(feature_indices, ply):
    """Returns (seg_sizes, per-core data dicts, samp_ids)."""
    fi = np.asarray(feature_indices, dtype=np.int64)
    plyv = np.asarray(ply, dtype=np.int64)
    bucket = np.clip(plyv // BUCKET_SIZE, 0, COUNT - 1)

    samp_ids = []
    core_ents = []  # per core: list over blocks of (idx_h0, own_h0, idx_h1, own_h1)
    nmax = 0
    for k in range(COUNT):
        samp = np.nonzero(bucket == k)[0]
        samp_ids.append(samp)
        nmax = max(nmax, len(samp))
    nblocks = max(1, -(-nmax // 128))

    for k in range(COUNT):
        samp = samp_ids[k]
        n = len(samp)
        rows = fi[samp]  # [n, 32]
        blocks = []
        for b in range(nblocks):
            lo, hi = b * 128, min(n, b * 128 + 128)
            if hi > lo:
                r = rows[lo:hi]
                owner = np.repeat(np.arange(hi - lo), NFEAT)
                rflat = r.reshape(-1)
                half = rflat >= NH
                per_half = []
                for h in (0, 1):
                    sel = np.nonzero(half == bool(h))[0]
                    per_half.append((rflat[sel] - h * NH,
                                     owner[sel].astype(np.float32)))
                blocks.append(per_half)
            else:
                z = (np.zeros(0, np.int64), np.zeros(0, np.float32))
                blocks.append([z, z])
        core_ents.append(blocks)

    # static segment sizes: max over cores, rounded up to 128
    seg_sizes = []
    for b in range(nblocks):
        for h in (0, 1):
            m = max(len(core_ents[k][b][h][0]) for k in range(COUNT))
            seg_sizes.append(-(-m // 128) * 128)

    calls = _plan_layout(seg_sizes)
    ncalls = len(calls)
    total_icol = sum(c[2] for c in calls) // 16
    total_chunks = sum(c[2] for c in calls) // 128
    import ml_dtypes
    iota = np.broadcast_to(np.arange(128, dtype=np.float32),
                           (128, 128)).astype(ml_dtypes.bfloat16)

    zeros = np.zeros((128, SUBCAP // 128, ODIM), ml_dtypes.bfloat16)
    cores = []
    for k in range(COUNT):
        idx_arr = np.full((128, max(16, total_icol)), -1, np.int16)
        own_arr = np.full((128, max(8, total_chunks)), -1.0,
                          ml_dtypes.bfloat16)
        cnt_arr = np.zeros((1, max(32, ncalls)), np.int32)
        for ci, (b, h, size, io, co, q) in enumerate(calls):
            ents, owns = core_ents[k][b][h]
            seg_off = 0
            # offset of this call within its (b,h) segment
            for cj in range(ci - 1, -1, -1):
                pb, ph, psz, _, _, _ = calls[cj]
                if pb == b and ph == h:
                    seg_off += psz
                else:
                    break
            seg = ents[seg_off:seg_off + size]
            osg = owns[seg_off:seg_off + size]
            m = len(seg)
            cnt_arr[0, ci] = m
            if m == 0:
                continue
            col = np.full(size, -1, np.int64)
            ocol = np.full(size, -1.0, np.float32)
            col[:m] = seg
            ocol[:m] = osg
            wrap = col.reshape(size // 16, 16).T.astype(np.int16)
            idx_arr[:, io:io + size // 16] = np.tile(wrap, (8, 1))
            own_arr[:, co:co + size // 128] = \
                ocol.reshape(size // 128, 128).T.astype(ml_dtypes.bfloat16)
        cores.append({"idxs": idx_arr, "owners": own_arr, "counts": cnt_arr,
                      "zeros": zeros, "iota": iota})
    return seg_sizes, cores, samp_ids


def _fallback(feature_indices, ply, W, bias):
    fi = np.asarray(feature_indices, dtype=np.int64)
    plyv = np.asarray(ply, dtype=np.int64)
    bucket = np.clip(plyv // BUCKET_SIZE, 0, COUNT - 1)
    Wr = np.asarray(W, dtype=np.float32).reshape(NROWS, COUNT, ODIM)
    br = np.asarray(bias, np.float32).reshape(COUNT, ODIM)
    out = np.empty((len(plyv), ODIM), np.float32)
    for b in range(len(plyv)):
        acc = Wr[fi[b], bucket[b], :].sum(axis=0) + br[bucket[b]]
        out[b] = np.clip(acc, 0.0, 1.0) ** 2 * ACT_SCALE
    return out


def _make_in_maps(cores, W, bias):
    import ml_dtypes
    wflat = W.reshape(-1).astype(ml_dtypes.bfloat16)
    biasr = bias.reshape(COUNT, ODIM)
    in_maps = []
    for k in range(COUNT):
        m = dict(cores[k])
        m["w"] = wflat[k * ODIM: k * ODIM + W_LEN]
        m["biasrep"] = np.broadcast_to(biasr[k], (128, ODIM)).copy()
        in_maps.append(m)
    return in_maps


def kernel(feature_indices, ply, W, bias):
    global _compiled
    from concourse.bass_utils import run_bass_kernel_spmd

    W = np.ascontiguousarray(np.asarray(W, dtype=np.float32))
    bias = np.asarray(bias, dtype=np.float32)
    seg_sizes, cores, samp_ids = _host_prep(feature_indices, ply)
    if max(len(s) for s in samp_ids) > 4096:
        return _fallback(feature_indices, ply, W, bias)

    calls = _plan_layout(seg_sizes)
    full_flags = tuple(
        all(int(c["counts"][0, ci]) == calls[ci][2] for c in cores)
        for ci in range(len(calls)))
    key = (tuple(seg_sizes), full_flags)
    if _compiled is None or _compiled[1] != key:
        _compiled = (_build_program(seg_sizes, full_flags), key)
    nc = _compiled[0]
    in_maps = _make_in_maps(cores, W, bias)
    res = run_bass_kernel_spmd(nc, in_maps, core_ids=list(range(COUNT)))
    out = np.empty((BATCH, ODIM), np.float32)
    for k in range(COUNT):
        ids = samp_ids[k]
        out[ids] = res.results[k]["out"][: len(ids)]
    return out


# revision 36
# speedup vs baseline: 1.1555x; 1.0574x over previous
"""Trainium2 Bass kernel for nn_PhaseAdaptiveInput (embedding lookup).

out[b] = act(sum_f W[feature_indices[b,f], bucket(b)*256:(bucket(b)+1)*256] + bias_bucket)
with bucket(b) = ply[b] // 7 and act(x) = clip(x,0,1)^2 * 255/256.

Strategy (8 NeuronCores, data parallel over samples, bucket-sharded):
  - Samples grouped by bucket host-side; core k gets bucket k's samples
    (~1024). All cores run ONE static SPMD program; the per-core bucket
    enters via the W input, a numpy VIEW of the (host-converted bf16) flat
    table starting at element 256*k. bf16 halves the gather bytes; rel err
    stays ~5e-3 (budget 2e-2).
  - Gathers are organized as per-(128-sample block, table half) segments,
    statically sized to the max count across the 8 cores (rounded to 128),
    split into near-uniform sub-calls of <= 896 indices (single_packet
    requires <= 63 descriptors/engine). Sub-calls are issued round-robin on
    4 SWDGE queues so descriptor generation runs concurrently on 4 Q7 core
    pairs (the single-queue Q7 descgen rate of ~8.5 ns/static-idx is the
    bottleneck otherwise; per-call cost is linear in static num_idxs, so
    bigger calls do NOT amortize).
  - Rows split by half (row < 32768 vs >= 32768) because gather indices are
    signed int16; the upper half uses a W view offset by 32768 rows.
  - Calls that are full on every core pass num_idxs_reg as an immediate;
    only segment-tail calls need a value_load of the per-core count (the
    count MUST match the Q7-stripped trailing -1 run or ring bookkeeping
    drifts).
  - Per 128-slot chunk a 0/1 bf16 mask [slot, sample] is built on DVE
    (owner-id vs iota is_equal) and chunk matmuls (bf16 -> f32 PSUM)
    accumulate per-sample sums into a per-block PSUM tile.
  - Epilogue per block: +bias on DVE, then clip(x,0,1) = relu(x)-relu(x-1)
    and square*255/256 (= Square(x*sqrt(s))) on the ACT engine.

The program is compiled on first call, specialized to the input's segment
sizes; recompiled only if a later input changes the layout.

Self-contained: hardcodes all shapes for the 8192x32 / 65536x2048 problem.
"""
import sys
import numpy as np

for _p in ("/opt/trn_rl_repo", "/root/.axon_site/_ro/trn_rl_repo"):
    if _p not in sys.path:
        sys.path.append(_p)

# ---------------------------------------------------------------- constants
BATCH = 8192
NFEAT = 32
NROWS = 65536
COUNT = 8
ODIM = 256
BUCKET_SIZE = 7
ACT_SCALE = 255.0 / 256.0
ROW_STRIDE = 2048          # f32 elements per table row
NH = 32768                 # rows per int16-addressable half
SUBCAP = 896               # max idxs per gather sub-call (single_packet: <=63 descs/engine)
NQUEUES = 4
W_LEN = (2 * NH - 1) * ROW_STRIDE + ODIM   # per-core view length
GATHER_BUFS = 16
MASK_BUFS = 10

_compiled = None           # (nc, layout)


def _plan_layout(seg_sizes):
    """seg_sizes: list over (block, half) in stream order of static sizes
    (multiples of 128, possibly 0). Returns the static call plan."""
    calls = []  # (block, half, size, icol_off, chunk_off, queue)
    icol_off = 0
    chunk_off = 0
    ci = 0
    for b in range(len(seg_sizes) // 2):
        for h in (0, 1):
            seg = seg_sizes[2 * b + h]
            if seg == 0:
                continue
            # near-uniform sub-call sizes (multiples of 128) so concurrent
            # queue groups don't wait on one long straggler call
            n_sub = -(-seg // SUBCAP)
            units = seg // 128
            per, rem = divmod(units, n_sub)
            sizes = [(per + 1) * 128] * rem + [per * 128] * (n_sub - rem)
            for size in sizes:
                calls.append((b, h, size, icol_off, chunk_off, ci % NQUEUES))
                icol_off += size // 16
                chunk_off += size // 128
                ci += 1
    return calls


def _build_program(seg_sizes, full_flags):
    import concourse.bacc as bacc
    import concourse.bass as bass
    import concourse.mybir as mybir
    import concourse.tile as tile
    from concourse.library_config import mlp

    F32 = mybir.dt.float32
    BF16 = mybir.dt.bfloat16

    calls = _plan_layout(seg_sizes)
    nblocks = len(seg_sizes) // 2
    ncalls = len(calls)
    total_icol = sum(c[2] for c in calls) // 16
    total_chunks = sum(c[2] for c in calls) // 128
    # first/last chunk index per block (for PSUM start/stop)
    blk_first = {}
    blk_last = {}
    for b, h, size, io, co, q in calls:
        for j in range(size // 128):
            ch = co + j
            blk_first.setdefault(b, ch)
            blk_last[b] = ch
    used_blocks = sorted(blk_first)

    nc = bacc.Bacc("TRN2", target_bir_lowering=False, debug=False,
                   num_swdge_queues=NQUEUES)
    w = nc.dram_tensor("w", [W_LEN], BF16, kind="ExternalInput")
    idxs_d = nc.dram_tensor("idxs", [128, max(16, total_icol)],
                            mybir.dt.int16, kind="ExternalInput")
    owners_d = nc.dram_tensor("owners", [128, max(8, total_chunks)], BF16,
                              kind="ExternalInput")
    counts_d = nc.dram_tensor("counts", [1, max(32, ncalls)], mybir.dt.int32,
                              kind="ExternalInput")
    zeros_d = nc.dram_tensor("zeros", [128, SUBCAP // 128, ODIM], BF16,
                             kind="ExternalInput")
    bias_d = nc.dram_tensor("biasrep", [128, ODIM], F32, kind="ExternalInput")
    iota_d = nc.dram_tensor("iota", [128, 128], BF16, kind="ExternalInput")
    out_d = nc.dram_tensor("out", [nblocks * 128, ODIM], F32,
                           kind="ExternalOutput")
    wt = w[:].tensor

    with tile.TileContext(nc) as tc:
        with tc.tile_pool(name="const", bufs=1) as cpool, \
             tc.tile_pool(name="gather", bufs=GATHER_BUFS) as gpool, \
             tc.tile_pool(name="mask", bufs=MASK_BUFS) as mpool, \
             tc.tile_pool(name="acts", bufs=2) as apool, \
             tc.tile_pool(name="psum", bufs=4, space="PSUM") as pspool:
            nc.gpsimd.load_library(mlp)
            idx_t = cpool.tile([128, max(16, total_icol)], mybir.dt.int16,
                               tag="idx")
            own_t = cpool.tile([128, max(8, total_chunks)], BF16, tag="own")
            cnt_t = cpool.tile([1, max(32, ncalls)], mybir.dt.int32,
                               tag="cnt")
            bias_t = cpool.tile([128, ODIM], F32, tag="bias")
            iota_t = cpool.tile([128, 128], BF16, tag="iota")
            nc.sync.dma_start(idx_t[:, :], idxs_d[:, :])
            nc.sync.dma_start(own_t[:, :], owners_d[:, :])
            nc.sync.dma_start(cnt_t[:, :], counts_d[:, :])
            nc.sync.dma_start(bias_t[:, :], bias_d[:, :])
            nc.sync.dma_start(iota_t[:, :], iota_d[:, :])
            zero_s = cpool.tile([128, 1], F32, tag="zs")
            negone_s = cpool.tile([128, 1], F32, tag="ns")
            nc.vector.memset(zero_s[:], 0.0)
            nc.vector.memset(negone_s[:], -1.0)

            psum_tiles = {}
            ncold = min(GATHER_BUFS, ncalls)
            for ci, (b, h, size, io, co, q) in enumerate(calls):
                schunk = size // 128
                dst = gpool.tile([128, SUBCAP // 128, ODIM], BF16, tag="dst")
                if ci < ncold and not full_flags[ci]:
                    # first use of this pool buffer gathers fewer rows than
                    # the static size: zero-fill via HWDGE DMA so the unset
                    # slots hold 0.0 rather than uninitialized SBUF (0 x NaN
                    # would poison the matmul). Later reuses inherit valid
                    # bf16 table values, so only first uses need this.
                    nc.sync.dma_start(dst[:], zeros_d[:, :, :])
                if full_flags[ci]:
                    # call is full on every core: immediate count, no
                    # value_load in the dispatch stream
                    cnt = size
                else:
                    cnt = nc.gpsimd.value_load(cnt_t[:1, ci:ci + 1])
                w_view = bass.AP(tensor=wt, offset=h * NH * ROW_STRIDE,
                                 ap=[(ROW_STRIDE, NH), (1, ODIM)])
                nc.gpsimd.dma_gather(
                    dst[:, :schunk, :], w_view,
                    idx_t[:, io:io + size // 16],
                    size, cnt, ODIM, elem_step=ROW_STRIDE,
                    single_packet=True, queue_num=q)

                mask = mpool.tile([128, SUBCAP // 128, 128], BF16, tag="mask")
                own_bc = own_t[:, co:co + schunk] \
                    .unsqueeze(2).to_broadcast([128, schunk, 128])
                iota_bc = iota_t[:, :].unsqueeze(1) \
                    .to_broadcast([128, schunk, 128])
                nc.vector.tensor_tensor(mask[:, :schunk, :], own_bc, iota_bc,
                                        mybir.AluOpType.is_equal)

                if blk_first[b] == co:
                    psum_tiles[b] = pspool.tile([128, ODIM], F32, tag="ps",
                                                name=f"ps{b}")
                pt = psum_tiles[b]
                for j in range(schunk):
                    ch = co + j
                    nc.tensor.matmul(pt[:],
                                     lhsT=mask[:, j, :],
                                     rhs=dst[:, j, :],
                                     start=(ch == blk_first[b]),
                                     stop=(ch == blk_last[b]))
                    if ch == blk_last[b]:
                        # clip(x,0,1) == relu(x) - relu(x-1); then
                        # square-and-scale via Square(d*sqrt(s)) = d^2*s.
                        # relu/square run on the (otherwise idle) ACT engine.
                        act = apool.tile([128, ODIM], F32, tag="act")
                        r1 = apool.tile([128, ODIM], F32, tag="r1")
                        nc.vector.tensor_add(act[:], pt[:], bias_t[:])
                        nc.scalar.activation(
                            r1[:], act[:],
                            mybir.ActivationFunctionType.Relu,
                            bias=negone_s[:, :1])
                        nc.scalar.activation(
                            act[:], act[:],
                            mybir.ActivationFunctionType.Relu,
                            bias=zero_s[:, :1])
                        nc.vector.tensor_sub(act[:], act[:], r1[:])
                        nc.scalar.activation(
                            act[:], act[:],
                            mybir.ActivationFunctionType.Square,
                            bias=zero_s[:, :1],
                            scale=float(ACT_SCALE ** 0.5))
                        nc.sync.dma_start(
                            out_d[b * 128:(b + 1) * 128, :], act[:])
            # blocks with zero static size still need defined output rows
            for b in range(nblocks):
                if b not in blk_first:
                    act = apool.tile([128, ODIM], F32, tag="act")
                    nc.vector.memset(act[:], 0.0)
                    nc.sync.dma_start(out_d[b * 128:(b + 1) * 128, :], act[:])
    nc.compile()
    return nc


def _host_prep(feature_indices, ply):
    """Returns (seg_sizes, per-core data dicts, samp_ids)."""
    fi = np.asarray(feature_indices, dtype=np.int64)
    plyv = np.asarray(ply, dtype=np.int64)
    bucket = np.clip(plyv // BUCKET_SIZE, 0, COUNT - 1)

    samp_ids = []
    core_ents = []  # per core: list over blocks of (idx_h0, own_h0, idx_h1, own_h1)
    nmax = 0
    for k in range(COUNT):
        samp = np.nonzero(bucket == k)[0]
        samp_ids.append(samp)
        nmax = max(nmax, len(samp))
    nblocks = max(1, -(-nmax // 128))

    for k in range(COUNT):
        samp = samp_ids[k]
        n = len(samp)
        rows = fi[samp]  # [n, 32]
        blocks = []
        for b in range(nblocks):
            lo, hi = b * 128, min(n, b * 128 + 128)
            if hi > lo:
                r = rows[lo:hi]
                owner = np.repeat(np.arange(hi - lo), NFEAT)
                rflat = r.reshape(-1)
                half = rflat >= NH
                per_half = []
                for h in (0, 1):
                    sel = np.nonzero(half == bool(h))[0]
                    per_half.append((rflat[sel] - h * NH,
                                     owner[sel].astype(np.float32)))
                blocks.append(per_half)
            else:
                z = (np.zeros(0, np.int64), np.zeros(0, np.float32))
                blocks.append([z, z])
        core_ents.append(blocks)

    # static segment sizes: max over cores, rounded up to 128
    seg_sizes = []
    for b in range(nblocks):
        for h in (0, 1):
            m = max(len(core_ents[k][b][h][0]) for k in range(COUNT))
            seg_sizes.append(-(-m // 128) * 128)

    calls = _plan_layout(seg_sizes)
    ncalls = len(calls)
    total_icol = sum(c[2] for c in calls) // 16
    total_chunks = sum(c[2] for c in calls) // 128
    import ml_dtypes
    iota = np.broadcast_to(np.arange(128, dtype=np.float32),
                           (128, 128)).astype(ml_dtypes.bfloat16)

    zeros = np.zeros((128, SUBCAP // 128, ODIM), ml_dtypes.bfloat16)
    cores = []
    for k in range(COUNT):
        idx_arr = np.full((128, max(16, total_icol)), -1, np.int16)
        own_arr = np.full((128, max(8, total_chunks)), -1.0,
                          ml_dtypes.bfloat16)
        cnt_arr = np.zeros((1, max(32, ncalls)), np.int32)
        for ci, (b, h, size, io, co, q) in enumerate(calls):
            ents, owns = core_ents[k][b][h]
            seg_off = 0
            # offset of this call within its (b,h) segment
            for cj in range(ci - 1, -1, -1):
                pb, ph, psz, _, _, _ = calls[cj]
                if pb == b and ph == h:
                    seg_off += psz
                else:
                    break
            seg = ents[seg_off:seg_off + size]
            osg = owns[seg_off:seg_off + size]
            m = len(seg)
            cnt_arr[0, ci] = m
            if m == 0:
                continue
            col = np.full(size, -1, np.int64)
            ocol = np.full(size, -1.0, np.float32)
            col[:m] = seg
            ocol[:m] = osg
            wrap = col.reshape(size // 16, 16).T.astype(np.int16)
            idx_arr[:, io:io + size // 16] = np.tile(wrap, (8, 1))
            own_arr[:, co:co + size // 128] = \
                ocol.reshape(size // 128, 128).T.astype(ml_dtypes.bfloat16)
        cores.append({"idxs": idx_arr, "owners": own_arr, "counts": cnt_arr,
                      "zeros": zeros, "iota": iota})
    return seg_sizes, cores, samp_ids


def _fallback(feature_indices, ply, W, bias):
    fi = np.asarray(feature_indices, dtype=np.int64)
    plyv = np.asarray(ply, dtype=np.int64)
    bucket = np.clip(plyv // BUCKET_SIZE, 0, COUNT - 1)
    Wr = np.asarray(W, dtype=np.float32).reshape(NROWS, COUNT, ODIM)
    br = np.asarray(bias, np.float32).reshape(COUNT, ODIM)
    out = np.empty((len(plyv), ODIM), np.float32)
    for b in range(len(plyv)):
        acc = Wr[fi[b], bucket[b], :].sum(axis=0) + br[bucket[b]]
        out[b] = np.clip(acc, 0.0, 1.0) ** 2 * ACT_SCALE
    return out


def _make_in_maps(cores, W, bias):
    import ml_dtypes
    wflat = W.reshape(-1).astype(ml_dtypes.bfloat16)
    biasr = bias.reshape(COUNT, ODIM)
    in_maps = []
    for k in range(COUNT):
        m = dict(cores[k])
        m["w"] = wflat[k * ODIM: k * ODIM + W_LEN]
        m["biasrep"] = np.broadcast_to(biasr[k], (128, ODIM)).copy()
        in_maps.append(m)
    return in_maps


def kernel(feature_indices, ply, W, bias):
    global _compiled
    from concourse.bass_utils import run_bass_kernel_spmd

    W = np.ascontiguousarray(np.asarray(W, dtype=np.float32))
    bias = np.asarray(bias, dtype=np.float32)
    seg_sizes, cores, samp_ids = _host_prep(feature_indices, ply)
    if max(len(s) for s in samp_ids) > 4096:
        return _fallback(feature_indices, ply, W, bias)

    calls = _plan_layout(seg_sizes)
    full_flags = tuple(
        all(int(c["counts"][0, ci]) == calls[ci][2] for c in cores)
        for ci in range(len(calls)))
    key = (tuple(seg_sizes), full_flags)
    if _compiled is None or _compiled[1] != key:
        _compiled = (_build_program(seg_sizes, full_flags), key)
    nc = _compiled[0]
    in_maps = _make_in_maps(cores, W, bias)
    res = run_bass_kernel_spmd(nc, in_maps, core_ids=list(range(COUNT)))
    out = np.empty((BATCH, ODIM), np.float32)
    for k in range(COUNT):
        ids = samp_ids[k]
        out[ids] = res.results[k]["out"][: len(ids)]
    return out
